# revision 38
# baseline (speedup 1.0000x reference)
"""Per-env MLP (EnvironVectorField) Trainium2 kernel, v4.

Reference computation (fp32):
    x = u.reshape(B, E, D)  # B=16384, E=8 envs, D=64
    h = swish(x @ W1[e] + b1[e]); h = swish(h @ W2[e] + b2[e])
    h = swish(h @ W3[e] + b3[e]); out = h @ W4[e] + b4[e]
    return out.reshape(B*E, D)

Sharding: expert-parallel — core e computes env e entirely (u rows e::8).

Design (all measured on this hardware):
- Serial K=128 bf16 matmuls stream at ~259 ns per 512 columns (~2.0 GHz
  effective). Row-tiled K=64 matmuls on opposite partition halves run
  CONCURRENTLY at ~106-120 ns each when (a) tile_position is constant
  within each PSUM accumulation group (mixing positions crashes the
  runtime) and (b) each bank's instruction cadence is >= 4 issue slots
  (back-to-back accumulation into one bank serializes on the ~128-cycle
  array drain). This kernel packs ALL layers that way: ~1.2x more
  MACs/s than the serial form.
- All device tensors are feature-major; the host ships x pre-transposed
  (D, B) and reads y back (D, B), so the PE does zero transposes.
- Everything computes in bf16 (fp32 PSUM accumulation). Host pre-casts.
- Mid layers: K=1024 as 16 K=64 slices. Per m-group, bank P accumulates
  the 8 partition-lo slices at (0,0), bank Q the partition-hi slices at
  (64,0). Two m-groups run interleaved (8 banks in flight, cadence 4).
  Combine + bias + swish: ACT tmp=Q+b, DVE P+=tmp, ACT hd=silu(P).
- L1 (K=64): two m-tiles concurrently via row tiles (0,0)/(64,0), with
  x duplicated into partitions 64..127.
- L4 (M=64): the two 256-col halves of each chunk run concurrently via
  column tiles (0,0)/(0,64) into one bank's partition halves.
- Batch processed in NB=512-column chunks; L1 of chunk c+1 runs between
  L3(c) and L4(c) to absorb evacuation latencies.
"""

import sys

sys.path.insert(0, '/opt/trn_rl_repo')

from contextlib import ExitStack

import ml_dtypes
import numpy as np

import concourse.bacc as bacc
import concourse.bass as bass
import concourse.mybir as mybir
import concourse.tile as tile
from concourse.bass_utils import run_bass_kernel_spmd

F32 = mybir.dt.float32
BF16 = mybir.dt.bfloat16
SILU = mybir.ActivationFunctionType.Silu
IDENT = mybir.ActivationFunctionType.Identity

N_ENV = 8
D = 64          # state dim
H = 1024        # hidden dim
B = 16384       # rows per env
NB = 512        # batch-chunk columns per chunk
NCH = B // NB   # chunks
KT = H // 128   # k/m tiles of 128 over the hidden dim


def build_module(iters: int = 1, phases: str = 'all'):
    nc = bacc.Bacc("TRN2", target_bir_lowering=False, num_devices=N_ENV)

    xin = nc.dram_tensor("x", (D, B), BF16, kind="ExternalInput")
    w1 = nc.dram_tensor("w1", (128, H), BF16, kind="ExternalInput")     # rows 0:64 = W1, 64:128 = W1
    w2 = nc.dram_tensor("w2", (128, KT, H), BF16, kind="ExternalInput")  # [ki, ko, M]
    w3 = nc.dram_tensor("w3", (128, KT, H), BF16, kind="ExternalInput")
    w4 = nc.dram_tensor("w4", (128, KT, D), BF16, kind="ExternalInput")
    b1 = nc.dram_tensor("b1", (128, KT), F32, kind="ExternalInput")      # [mi, mo]
    b2 = nc.dram_tensor("b2", (128, KT), F32, kind="ExternalInput")
    b3 = nc.dram_tensor("b3", (128, KT), F32, kind="ExternalInput")
    b4 = nc.dram_tensor("b4", (128, 1), F32, kind="ExternalInput")       # rows 0:64 = b4, 64:128 = b4
    yout = nc.dram_tensor("y", (D, B), F32, kind="ExternalOutput")

    xv = xin.rearrange("d (c n) -> c d n", n=NB)
    yv = yout.rearrange("d (c n) -> c d n", n=NB)

    with tile.TileContext(nc) as tc, ExitStack() as ctx:
        wpool = ctx.enter_context(tc.tile_pool(name="wpool", bufs=1))
        mps = ctx.enter_context(tc.tile_pool(name="mps", bufs=4, space="PSUM"))
        tmps = ctx.enter_context(tc.tile_pool(name="tmps", bufs=4))

        xT0 = wpool.tile([128, NB], BF16)
        xT1 = wpool.tile([128, NB], BF16)
        xT2 = wpool.tile([128, NB], BF16)
        xTs = (xT0, xT1, xT2)

        def dma_in(c, it=0):
            xT = xTs[c % 3]
            nc.sync.dma_start(xT[0:D, :], xv[c])
            nc.sync.dma_start(xT[D:128, :], xv[c])

        if iters == 1:
            # queue chunk-0/1 inputs ahead of the 4.5MB weight DMAs
            dma_in(0)
            dma_in(1)

        # biases in one padded tile
        ball = wpool.tile([128, 3 * KT + 1], F32)
        nc.sync.dma_start(ball[:, 0:KT], b1[:])
        nc.sync.dma_start(ball[:, KT:2 * KT], b2[:])
        nc.sync.dma_start(ball[:, 2 * KT:3 * KT], b3[:])
        nc.sync.dma_start(ball[:, 3 * KT:3 * KT + 1], b4[:])
        b1s = ball[:, 0:KT]
        b2s = ball[:, KT:2 * KT]
        b3s = ball[:, 2 * KT:3 * KT]
        b4s = ball[:, 3 * KT:3 * KT + 1]

        w1r = wpool.tile([128, H], BF16)
        w2r = wpool.tile([128, KT, H], BF16)
        w3r = wpool.tile([128, KT, H], BF16)
        w4r = wpool.tile([128, KT, D], BF16)
        # w2/w3 split in halves so they spread across DMA queues (a single
        # 2MB transfer gates L2(0) by ~12us at startup)
        nc.sync.dma_start(w1r[:], w1[:])
        nc.sync.dma_start(w2r[:, 0:KT // 2, :], w2[:, 0:KT // 2, :])
        nc.sync.dma_start(w2r[:, KT // 2:, :], w2[:, KT // 2:, :])
        nc.sync.dma_start(w3r[:, 0:KT // 2, :], w3[:, 0:KT // 2, :])
        nc.sync.dma_start(w3r[:, KT // 2:, :], w3[:, KT // 2:, :])
        nc.sync.dma_start(w4r[:], w4[:])

        # persistent activation buffers (fixed roles)
        hA0 = wpool.tile([128, KT, NB], BF16)  # L1 out, double-buffered
        hA1 = wpool.tile([128, KT, NB], BF16)
        hB = wpool.tile([128, KT, NB], BF16)   # L2 out
        hC = wpool.tile([128, KT, NB], BF16)   # L3 out
        oT0 = wpool.tile([D, NB], F32)         # L4 out
        oT1 = wpool.tile([D, NB], F32)

        def l1_pair(c, mp, it=0):
            # two m-tiles concurrently: row tiles (0,0) and (64,0), K=64 each
            xT = xTs[c % 3]
            hA = (hA0, hA1)[c % 2]
            mA, mB = 2 * mp, 2 * mp + 1
            pa = mps.tile([128, NB], F32, tag="pP", name=f"p1a_{it}_{c}_{mp}")
            pb = mps.tile([128, NB], F32, tag="pQ", name=f"p1b_{it}_{c}_{mp}")
            nc.tensor.matmul(pa[:], w1r[0:D, mA * 128:(mA + 1) * 128],
                             xT[0:D, :], start=True, stop=True,
                             tile_position=(0, 0))
            nc.tensor.matmul(pb[:], w1r[D:128, mB * 128:(mB + 1) * 128],
                             xT[D:128, :], start=True, stop=True,
                             tile_position=(64, 0))
            nc.scalar.activation(hA[:, mA, :], pa[:], SILU, bias=b1s[:, mA:mA + 1])
            nc.scalar.activation(hA[:, mB, :], pb[:], SILU, bias=b1s[:, mB:mB + 1])

        def mid_pair(li, wr, bs, hs, hd, c, mp, it=0):
            # two m-groups interleaved; per group: bank P accumulates the 8
            # partition-lo K=64 slices at (0,0), bank Q the hi slices at
            # (64,0). Issue alternates lo/hi; per-bank cadence = 4 slots.
            ms = [2 * mp, 2 * mp + 1]
            pP = [mps.tile([128, NB], F32, tag="pP",
                           name=f"pP{li}_{it}_{c}_{m}")[:] for m in ms]
            pQ = [mps.tile([128, NB], F32, tag="pQ",
                           name=f"pQ{li}_{it}_{c}_{m}")[:] for m in ms]
            for j in range(KT):
                for g, m in enumerate(ms):
                    ms_ = slice(m * 128, (m + 1) * 128)
                    nc.tensor.matmul(pP[g], wr[0:64, j, ms_], hs[0:64, j, :],
                                     start=(j == 0), stop=(j == KT - 1),
                                     tile_position=(0, 0))
                    nc.tensor.matmul(pQ[g], wr[64:128, j, ms_], hs[64:128, j, :],
                                     start=(j == 0), stop=(j == KT - 1),
                                     tile_position=(64, 0))
            # tmp copies first, then adds, then silus: keeps the second tmp
            # from queueing behind the first silu in the ACT FIFO
            tmpt = []
            for g, m in enumerate(ms):
                tmp = tmps.tile([128, NB], F32, tag="t", name=f"t{li}_{it}_{c}_{m}")
                nc.scalar.activation(tmp[:], pQ[g], IDENT, bias=bs[:, m:m + 1])
                tmpt.append(tmp)
            for g, m in enumerate(ms):
                nc.vector.tensor_add(pP[g], pP[g], tmpt[g][:])
            for g, m in enumerate(ms):
                nc.scalar.activation(hd[:, m, :], pP[g], SILU)

        def tail(c, it=0):
            # L4: hC -> oT, two 256-col halves via column tiling; each half
            # additionally splits even/odd k across two half-banks so the
            # per-bank cadence is 4 slots (no drain serialization).
            # Combine with two DVE adds (only one PSUM operand per op).
            oT = (oT0, oT1)[c % 2]
            hw = NB // 2
            # both tail banks from one tag: keeps per-chunk allocation counts
            # EVEN on each tag so the 4-generation rotation never desyncs
            p4a = mps.tile([128, hw], F32, tag="pP", name=f"p4a_{it}_{c}")
            p4b = mps.tile([128, hw], F32, tag="pP", name=f"p4b_{it}_{c}")
            for ko in range(KT // 2):
                for q, p4 in ((0, p4a), (1, p4b)):
                    k = 2 * ko + q
                    nc.tensor.matmul(p4[0:D, :], w4r[:, k, :], hC[:, k, 0:hw],
                                     start=(ko == 0), stop=(ko == KT // 2 - 1),
                                     tile_position=(0, 0))
                    nc.tensor.matmul(p4[D:128, :], w4r[:, k, :], hC[:, k, hw:NB],
                                     start=(ko == 0), stop=(ko == KT // 2 - 1),
                                     tile_position=(0, 64))
            nc.vector.tensor_scalar_add(oT[:, 0:hw], p4a[0:D, :], b4s[0:D])
            nc.vector.tensor_scalar_add(oT[:, hw:NB], p4a[D:128, :], b4s[D:128])
            nc.vector.tensor_add(oT[:, 0:hw], oT[:, 0:hw], p4b[0:D, :])
            nc.vector.tensor_add(oT[:, hw:NB], oT[:, hw:NB], p4b[D:128, :])
            nc.sync.dma_start(yv[c], oT[:])

        if phases == 'mids':
            nc.any.memzero(hA0[:])
            nc.any.memzero(hA1[:])

        def full_pass(it=0):
            if iters != 1:
                dma_in(0, it)
                dma_in(1, it)
            if phases != 'mids':
                for mp in range(KT // 2):
                    l1_pair(0, mp, it)
            for c in range(NCH):
                if c + 2 < NCH:
                    dma_in(c + 2, it)
                for mp in range(KT // 2):
                    mid_pair(2, w2r, b2s, (hA0, hA1)[c % 2], hB, c, mp, it)
                    # ONE long insertion (~2.7us) after mp1: tail(c-1) (its
                    # hC reads are long ready) plus ALL of L1(c+1) (hA is
                    # double-buffered, so no WAR with L2(c)'s reads). A
                    # single block longer than the ~1.5us evac-chain latency
                    # keeps every PSUM generation reuse out of the stall
                    # window; short scattered blocks each cost ~0.5-1.2us.
                    if mp == 1 and phases != 'mids':
                        if c > 0:
                            tail(c - 1, it)
                        if c + 1 < NCH:
                            for lp in range(KT // 2):
                                l1_pair(c + 1, lp, it)
                for mp in range(KT // 2):
                    mid_pair(3, w3r, b3s, hB, hC, c, mp, it)
                if phases == 'mids':
                    if c == NCH - 1:
                        tail(c, it)
                    continue
            if phases != 'mids':
                tail(NCH - 1, it)

        if iters == 1:
            full_pass()
        else:
            with tc.For_i(0, iters, 1):
                full_pass()

    nc.compile()
    return nc


def _prep_in_maps(t, u, W1, b1, W2, b2, W3, b3, W4, b4):
    bf = ml_dtypes.bfloat16
    u3 = np.asarray(u, np.float32).reshape(B, N_ENV, D)
    in_maps = []
    for e in range(N_ENV):
        w1p = np.empty((128, H), bf)
        w1p[:D] = W1[e].astype(bf)
        w1p[D:] = W1[e].astype(bf)
        b4p = np.empty((128, 1), np.float32)
        b4p[:D, 0] = b4[e]
        b4p[D:, 0] = b4[e]
        in_maps.append({
            "x": u3[:, e, :].T.astype(bf),
            "w1": w1p,
            "w2": W2[e].reshape(KT, 128, H).transpose(1, 0, 2).astype(bf),
            "w3": W3[e].reshape(KT, 128, H).transpose(1, 0, 2).astype(bf),
            "w4": W4[e].reshape(KT, 128, D).transpose(1, 0, 2).astype(bf),
            "b1": np.ascontiguousarray(b1[e].reshape(KT, 128).T.astype(np.float32)),
            "b2": np.ascontiguousarray(b2[e].reshape(KT, 128).T.astype(np.float32)),
            "b3": np.ascontiguousarray(b3[e].reshape(KT, 128).T.astype(np.float32)),
            "b4": b4p,
        })
    return in_maps


_CACHED_NC = None


def kernel(t, u, W1, b1, W2, b2, W3, b3, W4, b4):
    global _CACHED_NC
    u = np.asarray(u, np.float32)
    args = [np.asarray(a, np.float32) for a in (W1, b1, W2, b2, W3, b3, W4, b4)]
    if _CACHED_NC is None:
        _CACHED_NC = build_module()
    in_maps = _prep_in_maps(None, u, *args)
    res = run_bass_kernel_spmd(_CACHED_NC, in_maps, core_ids=list(range(N_ENV)))
    out = np.empty((B * N_ENV, D), np.float32)
    for e in range(N_ENV):
        out[e::N_ENV] = np.asarray(res.results[e]["y"], np.float32).T
    return out


# revision 39
# speedup vs baseline: 1.1982x; 1.1982x over previous
"""Per-env MLP (EnvironVectorField) Trainium2 kernel, v4.

Reference computation (fp32):
    x = u.reshape(B, E, D)  # B=16384, E=8 envs, D=64
    h = swish(x @ W1[e] + b1[e]); h = swish(h @ W2[e] + b2[e])
    h = swish(h @ W3[e] + b3[e]); out = h @ W4[e] + b4[e]
    return out.reshape(B*E, D)

Sharding: expert-parallel — core e computes env e entirely (u rows e::8).

Design (all measured on this hardware):
- Serial K=128 bf16 matmuls stream at ~259 ns per 512 columns (~2.0 GHz
  effective). Row-tiled K=64 matmuls on opposite partition halves run
  CONCURRENTLY at ~106-120 ns each when (a) tile_position is constant
  within each PSUM accumulation group (mixing positions crashes the
  runtime) and (b) each bank's instruction cadence is >= 4 issue slots
  (back-to-back accumulation into one bank serializes on the ~128-cycle
  array drain). This kernel packs ALL layers that way: ~1.2x more
  MACs/s than the serial form.
- All device tensors are feature-major; the host ships x pre-transposed
  (D, B) and reads y back (D, B), so the PE does zero transposes.
- Everything computes in bf16 (fp32 PSUM accumulation). Host pre-casts.
- Mid layers: K=1024 as 16 K=64 slices. Per m-group, bank P accumulates
  the 8 partition-lo slices at (0,0), bank Q the partition-hi slices at
  (64,0). Two m-groups run interleaved (8 banks in flight, cadence 4).
  Combine + bias + swish: ACT tmp=Q+b, DVE P+=tmp, ACT hd=silu(P).
- L1 (K=64): two m-tiles concurrently via row tiles (0,0)/(64,0), with
  x duplicated into partitions 64..127.
- L4 (M=64): the two 256-col halves of each chunk run concurrently via
  column tiles (0,0)/(0,64) into one bank's partition halves.
- Batch processed in NB=512-column chunks; L1 of chunk c+1 runs between
  L3(c) and L4(c) to absorb evacuation latencies.
"""

import sys

sys.path.insert(0, '/opt/trn_rl_repo')

from contextlib import ExitStack

import ml_dtypes
import numpy as np

import concourse.bacc as bacc
import concourse.bass as bass
import concourse.mybir as mybir
import concourse.tile as tile
from concourse.bass_utils import run_bass_kernel_spmd

F32 = mybir.dt.float32
BF16 = mybir.dt.bfloat16
SILU = mybir.ActivationFunctionType.Silu
IDENT = mybir.ActivationFunctionType.Identity

N_ENV = 8
D = 64          # state dim
H = 1024        # hidden dim
B = 16384       # rows per env
NB = 512        # batch-chunk columns per chunk
NCH = B // NB   # chunks
KT = H // 128   # k/m tiles of 128 over the hidden dim


def build_module(iters: int = 1, phases: str = 'all'):
    nc = bacc.Bacc("TRN2", target_bir_lowering=False, num_devices=N_ENV)

    xin = nc.dram_tensor("x", (D, B), BF16, kind="ExternalInput")
    w1 = nc.dram_tensor("w1", (128, H), BF16, kind="ExternalInput")     # rows 0:64 = W1, 64:128 = W1
    w2 = nc.dram_tensor("w2", (128, KT, H), BF16, kind="ExternalInput")  # [ki, ko, M]
    w3 = nc.dram_tensor("w3", (128, KT, H), BF16, kind="ExternalInput")
    w4 = nc.dram_tensor("w4", (128, KT, D), BF16, kind="ExternalInput")
    b1 = nc.dram_tensor("b1", (128, KT), F32, kind="ExternalInput")      # [mi, mo]
    b2 = nc.dram_tensor("b2", (128, KT), F32, kind="ExternalInput")
    b3 = nc.dram_tensor("b3", (128, KT), F32, kind="ExternalInput")
    b4 = nc.dram_tensor("b4", (128, 1), F32, kind="ExternalInput")       # rows 0:64 = b4, 64:128 = b4
    yout = nc.dram_tensor("y", (D, B), F32, kind="ExternalOutput")

    xv = xin.rearrange("d (c n) -> c d n", n=NB)
    yv = yout.rearrange("d (c n) -> c d n", n=NB)

    with tile.TileContext(nc) as tc, ExitStack() as ctx:
        wpool = ctx.enter_context(tc.tile_pool(name="wpool", bufs=1))
        mps = ctx.enter_context(tc.tile_pool(name="mps", bufs=4, space="PSUM"))
        tmps = ctx.enter_context(tc.tile_pool(name="tmps", bufs=4))

        xT0 = wpool.tile([128, NB], BF16)
        xT1 = wpool.tile([128, NB], BF16)

        def dma_in(c, it=0):
            xT = (xT0, xT1)[c % 2]
            nc.sync.dma_start(xT[0:D, :], xv[c])
            nc.sync.dma_start(xT[D:128, :], xv[c])

        if iters == 1:
            # queue chunk-0 input ahead of the 4.5MB weight DMAs (FIFO queues)
            dma_in(0)

        # biases in one padded tile
        ball = wpool.tile([128, 3 * KT + 1], F32)
        nc.sync.dma_start(ball[:, 0:KT], b1[:])
        nc.sync.dma_start(ball[:, KT:2 * KT], b2[:])
        nc.sync.dma_start(ball[:, 2 * KT:3 * KT], b3[:])
        nc.sync.dma_start(ball[:, 3 * KT:3 * KT + 1], b4[:])
        b1s = ball[:, 0:KT]
        b2s = ball[:, KT:2 * KT]
        b3s = ball[:, 2 * KT:3 * KT]
        b4s = ball[:, 3 * KT:3 * KT + 1]

        w1r = wpool.tile([128, H], BF16)
        w2r = wpool.tile([128, KT, H], BF16)
        w3r = wpool.tile([128, KT, H], BF16)
        w4r = wpool.tile([128, KT, D], BF16)
        nc.sync.dma_start(w1r[:], w1[:])
        nc.sync.dma_start(w2r[:], w2[:])
        nc.sync.dma_start(w3r[:], w3[:])
        nc.sync.dma_start(w4r[:], w4[:])

        # persistent activation buffers (fixed roles)
        hA0 = wpool.tile([128, KT, NB], BF16)  # L1 out, double-buffered
        hA1 = wpool.tile([128, KT, NB], BF16)
        hB = wpool.tile([128, KT, NB], BF16)   # L2 out
        hC = wpool.tile([128, KT, NB], BF16)   # L3 out
        oT0 = wpool.tile([D, NB], F32)         # L4 out
        oT1 = wpool.tile([D, NB], F32)

        def l1_pair(c, mp, it=0):
            # two m-tiles concurrently: row tiles (0,0) and (64,0), K=64 each
            xT = (xT0, xT1)[c % 2]
            hA = (hA0, hA1)[c % 2]
            mA, mB = 2 * mp, 2 * mp + 1
            pa = mps.tile([128, NB], F32, tag="pP", name=f"p1a_{it}_{c}_{mp}")
            pb = mps.tile([128, NB], F32, tag="pQ", name=f"p1b_{it}_{c}_{mp}")
            nc.tensor.matmul(pa[:], w1r[0:D, mA * 128:(mA + 1) * 128],
                             xT[0:D, :], start=True, stop=True,
                             tile_position=(0, 0))
            nc.tensor.matmul(pb[:], w1r[D:128, mB * 128:(mB + 1) * 128],
                             xT[D:128, :], start=True, stop=True,
                             tile_position=(64, 0))
            nc.scalar.activation(hA[:, mA, :], pa[:], SILU, bias=b1s[:, mA:mA + 1])
            nc.scalar.activation(hA[:, mB, :], pb[:], SILU, bias=b1s[:, mB:mB + 1])

        def mid_pair(li, wr, bs, hs, hd, c, mp, it=0):
            # two m-groups interleaved; per group: bank P accumulates the 8
            # partition-lo K=64 slices at (0,0), bank Q the hi slices at
            # (64,0). Issue alternates lo/hi; per-bank cadence = 4 slots.
            ms = [2 * mp, 2 * mp + 1]
            pP = [mps.tile([128, NB], F32, tag="pP",
                           name=f"pP{li}_{it}_{c}_{m}")[:] for m in ms]
            pQ = [mps.tile([128, NB], F32, tag="pQ",
                           name=f"pQ{li}_{it}_{c}_{m}")[:] for m in ms]
            for j in range(KT):
                for g, m in enumerate(ms):
                    ms_ = slice(m * 128, (m + 1) * 128)
                    nc.tensor.matmul(pP[g], wr[0:64, j, ms_], hs[0:64, j, :],
                                     start=(j == 0), stop=(j == KT - 1),
                                     tile_position=(0, 0))
                    nc.tensor.matmul(pQ[g], wr[64:128, j, ms_], hs[64:128, j, :],
                                     start=(j == 0), stop=(j == KT - 1),
                                     tile_position=(64, 0))
            # tmp copies first, then adds, then silus: keeps the second tmp
            # from queueing behind the first silu in the ACT FIFO
            tmpt = []
            for g, m in enumerate(ms):
                tmp = tmps.tile([128, NB], F32, tag="t", name=f"t{li}_{it}_{c}_{m}")
                nc.scalar.activation(tmp[:], pQ[g], IDENT, bias=bs[:, m:m + 1])
                tmpt.append(tmp)
            for g, m in enumerate(ms):
                nc.vector.tensor_add(pP[g], pP[g], tmpt[g][:])
            for g, m in enumerate(ms):
                nc.scalar.activation(hd[:, m, :], pP[g], SILU)

        def tail(c, it=0):
            # L4: hC -> oT, two 256-col halves via column tiling; each half
            # additionally splits even/odd k across two half-banks so the
            # per-bank cadence is 4 slots (no drain serialization).
            # Combine with two DVE adds (only one PSUM operand per op).
            oT = (oT0, oT1)[c % 2]
            hw = NB // 2
            # both tail banks from one tag: keeps per-chunk allocation counts
            # EVEN on each tag so the 4-generation rotation never desyncs
            p4a = mps.tile([128, hw], F32, tag="pP", name=f"p4a_{it}_{c}")
            p4b = mps.tile([128, hw], F32, tag="pP", name=f"p4b_{it}_{c}")
            for ko in range(KT // 2):
                for q, p4 in ((0, p4a), (1, p4b)):
                    k = 2 * ko + q
                    nc.tensor.matmul(p4[0:D, :], w4r[:, k, :], hC[:, k, 0:hw],
                                     start=(ko == 0), stop=(ko == KT // 2 - 1),
                                     tile_position=(0, 0))
                    nc.tensor.matmul(p4[D:128, :], w4r[:, k, :], hC[:, k, hw:NB],
                                     start=(ko == 0), stop=(ko == KT // 2 - 1),
                                     tile_position=(0, 64))
            nc.vector.tensor_scalar_add(oT[:, 0:hw], p4a[0:D, :], b4s[0:D])
            nc.vector.tensor_scalar_add(oT[:, hw:NB], p4a[D:128, :], b4s[D:128])
            nc.vector.tensor_add(oT[:, 0:hw], oT[:, 0:hw], p4b[0:D, :])
            nc.vector.tensor_add(oT[:, hw:NB], oT[:, hw:NB], p4b[D:128, :])
            nc.sync.dma_start(yv[c], oT[:])

        if phases == 'mids':
            nc.any.memzero(hA0[:])
            nc.any.memzero(hA1[:])

        def full_pass(it=0):
            if iters != 1:
                dma_in(0, it)
            if phases != 'mids':
                for mp in range(KT // 2):
                    l1_pair(0, mp, it)
            for c in range(NCH):
                if c + 1 < NCH:
                    dma_in(c + 1, it)
                for mp in range(KT // 2):
                    mid_pair(2, w2r, b2s, (hA0, hA1)[c % 2], hB, c, mp, it)
                    # ONE long insertion (~2.7us) after mp1: tail(c-1) (its
                    # hC reads are long ready) plus ALL of L1(c+1) (hA is
                    # double-buffered, so no WAR with L2(c)'s reads). A
                    # single block longer than the ~1.5us evac-chain latency
                    # keeps every PSUM generation reuse out of the stall
                    # window; short scattered blocks each cost ~0.5-1.2us.
                    if mp == 1 and phases != 'mids':
                        if c > 0:
                            tail(c - 1, it)
                        if c + 1 < NCH:
                            for lp in range(KT // 2):
                                l1_pair(c + 1, lp, it)
                for mp in range(KT // 2):
                    mid_pair(3, w3r, b3s, hB, hC, c, mp, it)
                if phases == 'mids':
                    if c == NCH - 1:
                        tail(c, it)
                    continue
            if phases != 'mids':
                tail(NCH - 1, it)

        if iters == 1:
            full_pass()
        else:
            with tc.For_i(0, iters, 1):
                full_pass()

    nc.compile()
    return nc


def _prep_in_maps(t, u, W1, b1, W2, b2, W3, b3, W4, b4):
    bf = ml_dtypes.bfloat16
    u3 = np.asarray(u, np.float32).reshape(B, N_ENV, D)
    in_maps = []
    for e in range(N_ENV):
        w1p = np.empty((128, H), bf)
        w1p[:D] = W1[e].astype(bf)
        w1p[D:] = W1[e].astype(bf)
        b4p = np.empty((128, 1), np.float32)
        b4p[:D, 0] = b4[e]
        b4p[D:, 0] = b4[e]
        in_maps.append({
            "x": u3[:, e, :].T.astype(bf),
            "w1": w1p,
            "w2": W2[e].reshape(KT, 128, H).transpose(1, 0, 2).astype(bf),
            "w3": W3[e].reshape(KT, 128, H).transpose(1, 0, 2).astype(bf),
            "w4": W4[e].reshape(KT, 128, D).transpose(1, 0, 2).astype(bf),
            "b1": np.ascontiguousarray(b1[e].reshape(KT, 128).T.astype(np.float32)),
            "b2": np.ascontiguousarray(b2[e].reshape(KT, 128).T.astype(np.float32)),
            "b3": np.ascontiguousarray(b3[e].reshape(KT, 128).T.astype(np.float32)),
            "b4": b4p,
        })
    return in_maps


_CACHED_NC = None


def kernel(t, u, W1, b1, W2, b2, W3, b3, W4, b4):
    global _CACHED_NC
    u = np.asarray(u, np.float32)
    args = [np.asarray(a, np.float32) for a in (W1, b1, W2, b2, W3, b3, W4, b4)]
    if _CACHED_NC is None:
        _CACHED_NC = build_module()
    in_maps = _prep_in_maps(None, u, *args)
    res = run_bass_kernel_spmd(_CACHED_NC, in_maps, core_ids=list(range(N_ENV)))
    out = np.empty((B * N_ENV, D), np.float32)
    for e in range(N_ENV):
        out[e::N_ENV] = np.asarray(res.results[e]["y"], np.float32).T
    return out
